# revision 50
# baseline (speedup 1.0000x reference)
"""DoubleMaskedChamferDistance Trainium2 kernel.

Full inputs: video_feat [128,512,512] f32, lang_feat [128,64,512] f32,
mask_v [128,512] f32, mask_l [128,64] f32  ->  out [128] f32.

Sharding: data-parallel over batch B=128 across 8 cores (16 per core).

Math notes:
 - pd[v,l] = |v|^2 - 2 v.l + |l|^2 ; masked = pd + (1 - mask_v mask_l) * max(pd).
   Any constant M >= max(pd) shields identically (pd <= ~1400; M = 32768 = 2^15,
   exact in bf16/fp32). This removes the cross-batch/cross-core dependency.
 - Per batch pair, one PSUM accumulation in [(t,l), v] layout:
       psum[l,v] = -M*mask_l[l]*mask_v[v]  (one K=2 block-diag matmul per pair)
                 - 2*ab[l,v]               (4 bf16 matmuls per batch over d-chunks)
                 + a[v]                    (1 all-ones matmul per batch)
   and + (b[l] + M) is applied as the ACT bias at evacuation.
 - minsl = min over v: free-dim reduce of masked. minsv = min over l:
   PE-transpose masked to [v, (t,l)] strips, free-dim reduce.
 - Per-batch partition sums are deferred: minsv/minsl/mask columns are
   collected across the batch loop and reduced once at the end (ones-matmuls).
 - DMA head: video chunk DMAs are issued first (gpsimd SWDGE), lang is
   split per-pair and interleaved, masks ride the sync HWDGE queue, and
   identity generation is emitted between the first chunk issues.

Toolchain constraint honored throughout: every DMA instruction may carry at
most ONE semaphore wait, so DMAs only ever write fresh (never-recycled) tiles
and all data marshalling between tiles is done by compute engines.
"""

import numpy as np

import concourse.bass as bass
import concourse.mybir as mybir
import concourse.tile as tile
from concourse import bacc, masks
from concourse.bass_utils import run_bass_kernel_spmd

N_CORES = 8
B, TV, TL, D = 128, 512, 64, 512
B_LOC = B // N_CORES  # 16
M_CONST = 32768.0

F32 = mybir.dt.float32
BF16 = mybir.dt.bfloat16
AX = mybir.AxisListType


def _emit(nc, tc, ctx, video, lang, mask_v, mask_l, out):
    TT = mybir.AluOpType
    AF = mybir.ActivationFunctionType

    consts = ctx.enter_context(tc.tile_pool(name="consts", bufs=1))
    vnat = ctx.enter_context(tc.tile_pool(name="vnat", bufs=1))
    vT = ctx.enter_context(tc.tile_pool(name="vT", bufs=6))
    langfresh = ctx.enter_context(tc.tile_pool(name="langfresh", bufs=1))
    langp = ctx.enter_context(tc.tile_pool(name="langp", bufs=3))
    sqs = ctx.enter_context(tc.tile_pool(name="sqs", bufs=2))
    smalls = ctx.enter_context(tc.tile_pool(name="smalls", bufs=4))
    maskedp = ctx.enter_context(tc.tile_pool(name="maskedp", bufs=3))
    ps_vT = ctx.enter_context(tc.tile_pool(name="ps_vT", bufs=2, space="PSUM"))
    ps_main = ctx.enter_context(tc.tile_pool(name="ps_main", bufs=2, space="PSUM"))
    ps_small = ctx.enter_context(tc.tile_pool(name="ps_small", bufs=2, space="PSUM"))

    NP = B_LOC // 2  # batch pairs

    # ---- DMA issue order is the head-latency critical path: video first ----
    vchunks = []
    lang_pairs = [None] * NP

    def issue_vchunk(c):
        t = vnat.tile([128, 4, 512], BF16, tag=f"vch{c}")
        nc.gpsimd.dma_start(
            out=t[:], in_=video[c].rearrange("(s p) d -> p s d", p=128)
        )
        vchunks.append(t)

    def issue_lang(j0, nj):
        # one DMA covers pairs [j0, j0+nj): [128(two,l), nj, 512] bf16
        t = langfresh.tile([128, nj, 512], BF16, tag=f"lg{j0}")
        nc.gpsimd.dma_start(
            out=t[:],
            in_=lang[2 * j0 : 2 * (j0 + nj)].rearrange(
                "(j two) l d -> (two l) j d", two=2
            ),
        )
        for jj in range(nj):
            lang_pairs[j0 + jj] = t[:, jj]

    # SWDGE assigns issues to 8 rotating rings in order, and ring service
    # order is not FIFO across rings: anything issued late can see its
    # transfers deferred past the whole video stream. So: small early-needed
    # operands go first, then all 16 video chunks back-to-back. The negm
    # memset runs on gpsimd so its DMAs' wait resolves in-queue instead of
    # stalling descriptor generation on a cross-engine semaphore.
    negm = consts.tile([2, NP, 128], BF16)
    nc.gpsimd.memset(negm[:], 0.0)

    issue_vchunk(0)
    issue_vchunk(1)
    issue_lang(0, 4)
    issue_lang(4, 4)
    nc.gpsimd.dma_start(
        out=negm[0:1, :, 0:64],
        in_=mask_l.rearrange("(j two) l -> two j l", two=2)[0:1],
    )
    nc.gpsimd.dma_start(
        out=negm[1:2, :, 64:128],
        in_=mask_l.rearrange("(j two) l -> two j l", two=2)[1:2],
    )
    # mask_v in pair layout for the K=2 mask matmul: mv_pairs[t, j, v]
    # (bf16 cast -> gpsimd SWDGE)
    mv_pairs = consts.tile([2, NP, 512], BF16)
    nc.gpsimd.dma_start(
        out=mv_pairs[:], in_=mask_v.rearrange("(j two) v -> two j v", two=2)
    )
    mvn = consts.tile([2, NP, 512], BF16)
    nc.vector.tensor_scalar_mul(mvn[:], mv_pairs[:], -M_CONST)

    # identb on gpsimd (affine_select is gpsimd-only); identf derived from
    # it by a DVE cast copy so gpsimd only pays for one identity build
    identb = consts.tile([128, 128], BF16)
    masks.make_identity(nc, identb[:])
    identf = consts.tile([128, 128], F32)
    nc.vector.tensor_copy(identf[:], identb[:])

    for c in range(2, B_LOC):
        issue_vchunk(c)

    # masks ride the sync (HWDGE) queue in parallel with the video stream
    maskv_nat = consts.tile([B_LOC, 512], F32)
    nc.sync.dma_start(out=maskv_nat[:], in_=mask_v)
    maskl_pair_nat = consts.tile([NP, 128], F32)
    nc.sync.dma_start(
        out=maskl_pair_nat[:], in_=mask_l.rearrange("(j two) l -> j (two l)", two=2)
    )

    # ---- constants / mask prep (compute engines, under the DMA stream) ----
    ones128 = consts.tile([128, 1], F32)
    nc.vector.memset(ones128[:], 1.0)
    ones_mat = consts.tile([128, 64], BF16)
    nc.vector.memset(ones_mat[:], 1.0)
    ones_top = consts.tile([128, 1], F32)
    nc.vector.memset(ones_top[:], 0.0)
    nc.vector.memset(ones_top[0:64], 1.0)
    ones_bot = consts.tile([128, 1], F32)
    nc.vector.memset(ones_bot[:], 0.0)
    nc.vector.memset(ones_bot[64:128], 1.0)

    # mask_v columns for the final masked sums: [v%128, v//128, b]
    mvc_ps = ps_small.tile([128, 4, B_LOC], F32, tag="ps_sm")
    for s in range(4):
        nc.tensor.transpose(
            mvc_ps[:, s],
            maskv_nat[:, 128 * s : 128 * (s + 1)],
            identf[0:B_LOC, 0:B_LOC],
        )
    maskv_cols = consts.tile([128, 4, B_LOC], F32)
    nc.vector.tensor_copy(maskv_cols[:], mvc_ps[:])

    # masklT_pair[(two l), j] = mask_l[2j + two, l]
    mlc_ps = ps_small.tile([128, NP], F32, tag="ps_sm")
    nc.tensor.transpose(mlc_ps[:], maskl_pair_nat[:], identf[0:NP, 0:NP])
    masklT_pair = consts.tile([128, NP], F32)
    nc.vector.tensor_copy(masklT_pair[:], mlc_ps[:])

    # collectors (written per pair/batch, reduced once at the end).
    # minsv_all is s-major [v%128, v//128, b] so the per-pair reduce's
    # output slice is packed (keeps the DVE 2x perf mode).
    minsv_all = consts.tile([128, 4, B_LOC], BF16)
    minsl_pairs = consts.tile([128, NP], F32)
    b_pairs = consts.tile([128, NP], F32)
    bias_pairs = consts.tile([128, NP], F32)

    # ---- all lang-side prep happens up front, in the window where the PE
    # is otherwise idle waiting for the first video chunks ----
    langTs = []
    for j in range(NP):
        lg = lang_pairs[j]
        sq_l = sqs.tile([128, 512], BF16, tag="sq_l")
        nc.scalar.activation(
            sq_l[:], lg[:], AF.Square, accum_out=b_pairs[:, j : j + 1]
        )
        lg_ps = ps_small.tile([128, 4, 128], BF16, tag="ps_sm")
        for k in range(4):
            nc.tensor.transpose(
                lg_ps[:, k], lg[:, 128 * k : 128 * (k + 1)], identb[:]
            )
        langT = langp.tile([128, 4, 128], BF16, tag=f"langT{j}")
        nc.vector.tensor_scalar_mul(langT[:], lg_ps[:], -2.0)
        langTs.append(langT)
    nc.vector.tensor_scalar_add(bias_pairs[:], b_pairs[:], M_CONST)

    # Software pipeline, one pair deep: prep(j) = PE transposes + DVE/ACT
    # evacuations + squares; compute(j) = matmuls + masked evac + mins.
    # Emitting compute(j-1) AFTER prep(j) keeps the in-order PE queue from
    # head-blocking on pair j-1's DVE/ACT chain: the PE runs pair j's
    # transposes while pair j-1's sq1/langT dependencies settle.
    prepped = {}

    def prep(j):
        vt_sbs, sq1s = [], []
        for t in range(2):
            vstrip = vchunks[2 * j + t]  # [128, 4, 512] bf16 (p, s, d)
            # videoT transposes; evacuations split DVE/ACT
            vt_sb = vT.tile([128, 4, 512], BF16, tag="vt_sb")
            vt_ps = ps_vT.tile([128, 4, 512], BF16, tag="vt_ps")
            for k in range(4):
                for s in range(4):
                    nc.tensor.transpose(
                        vt_ps[:, k, 128 * s : 128 * (s + 1)],
                        vstrip[:, s, 128 * k : 128 * (k + 1)],
                        identb[:],
                    )
            nc.scalar.copy(vt_sb[:], vt_ps[:])

            # a-chunks: vt^2 pre-added to one [128,512] slab (DVE).
            # (tensor_tensor may read at most ONE operand from PSUM, so the
            # squares read the SBUF copy.)
            sq4 = sqs.tile([128, 4, 512], BF16, tag="sq4")
            nc.vector.tensor_tensor(sq4[:], vt_sb[:], vt_sb[:], op=TT.mult)
            sq2 = sqs.tile([128, 2, 512], BF16, tag="sq2")
            nc.vector.tensor_tensor(sq2[:], sq4[:, 0:2], sq4[:, 2:4], op=TT.add)
            sq1 = sqs.tile([128, 512], BF16, tag=f"sq1_{t}")
            nc.vector.tensor_tensor(sq1[:], sq2[:, 0], sq2[:, 1], op=TT.add)
            vt_sbs.append(vt_sb)
            sq1s.append(sq1)
        prepped[j] = (vt_sbs, sq1s)

    def compute(j):
        vt_sbs, sq1s = prepped.pop(j)
        langT = langTs[j]
        # PSUM: per-half ab (k=0 opens) + a, then one K=2 mask matmul
        # (full pair width) closes the accumulation
        psum_pair = ps_main.tile([128, 512], F32, tag="psum_T")
        for t in range(2):
            half = psum_pair[64 * t : 64 * (t + 1), :]
            vt_sb = vt_sbs[t]
            for k in range(4):
                nc.tensor.matmul(
                    half,
                    langT[:, k, 64 * t : 64 * (t + 1)],
                    vt_sb[:, k],
                    start=(k == 0),
                    stop=False,
                    skip_group_check=True,
                )
            nc.tensor.matmul(
                half,
                ones_mat[:],
                sq1s[t][:],
                start=False,
                stop=False,
                skip_group_check=True,
            )
        nc.tensor.matmul(
            psum_pair[:],
            negm[:, j],
            mvn[:, j],
            start=False,
            stop=True,
            skip_group_check=True,
        )

        # masked evacuation with +(b + M) bias (bf16), both batches
        masked_pr = maskedp.tile([128, 512], BF16, tag="masked_pr")
        nc.scalar.activation(
            masked_pr[:],
            psum_pair[:],
            AF.Identity,
            bias=bias_pairs[:, j : j + 1],
            scale=1.0,
        )

        # minsl: min over v (free dim), both batches at once
        nc.vector.tensor_reduce(
            minsl_pairs[:, j : j + 1], masked_pr[:], axis=AX.X, op=TT.min
        )

        # minsv: transpose [128,128] pair-blocks, min over l
        o2 = ps_small.tile([128, 4, 2, 64], BF16, tag="ps_sm")
        for s in range(4):
            nc.tensor.transpose(
                o2[:, s], masked_pr[:, 128 * s : 128 * (s + 1)], identb[:]
            )
        nc.vector.tensor_reduce(
            minsv_all[:, :, 2 * j : 2 * j + 2], o2[:], axis=AX.X, op=TT.min
        )

    for j in range(NP):
        prep(j)
        if j >= 1:
            compute(j - 1)
    compute(NP - 1)

    # ---- final: masked sums via ones-matmuls over collected columns ----
    mv_mask = consts.tile([128, 4, B_LOC], F32)
    nc.vector.tensor_tensor(mv_mask[:], minsv_all[:], maskv_cols[:], op=TT.mult)
    mv_sums = consts.tile([128, B_LOC], F32)
    nc.vector.tensor_reduce(
        mv_sums[:], mv_mask[:].rearrange("p s b -> p b s"), axis=AX.X, op=TT.add
    )
    nv_sums = consts.tile([128, B_LOC], F32)
    nc.vector.tensor_reduce(
        nv_sums[:],
        maskv_cols[:].rearrange("p s b -> p b s"),
        axis=AX.X,
        op=TT.add,
    )
    mlm = consts.tile([128, NP], F32)
    nc.vector.tensor_tensor(mlm[:], minsl_pairs[:], masklT_pair[:], op=TT.mult)

    red_mv = ps_main.tile([1, B_LOC], F32, tag="psum_T")
    red_nv = ps_small.tile([1, B_LOC], F32, tag="ps_sm")
    nc.tensor.matmul(red_mv[:], ones128[:], mv_sums[:], start=True, stop=True)
    nc.tensor.matmul(red_nv[:], ones128[:], nv_sums[:], start=True, stop=True)
    rv = smalls.tile([1, B_LOC], F32, tag="rv")
    t1 = smalls.tile([1, B_LOC], F32, tag="t1")
    nc.vector.reciprocal(rv[:], red_nv[:])
    nc.vector.tensor_tensor(t1[:], red_mv[:], rv[:], op=TT.mult)

    # even/odd batch reductions as separate partition-0 matmuls, written
    # into the interleaved positions of t2 via strided views
    t2 = smalls.tile([1, B_LOC], F32, tag="t2")
    t2v = t2[:].rearrange("a (jj two) -> a jj two", two=2)
    rl = smalls.tile([1, NP], F32, tag="rl")

    red_ml_e = ps_main.tile([1, NP], F32, tag="psum_T")
    red_nl_e = ps_small.tile([1, NP], F32, tag="ps_sm")
    nc.tensor.matmul(red_ml_e[:], ones_top[:], mlm[:], start=True, stop=True)
    nc.tensor.matmul(
        red_nl_e[:], ones_top[:], masklT_pair[:], start=True, stop=True
    )
    nc.vector.reciprocal(rl[:], red_nl_e[:])
    nc.vector.tensor_tensor(t2v[:, :, 0], red_ml_e[:], rl[:], op=TT.mult)

    red_ml_o = ps_main.tile([1, NP], F32, tag="psum_T")
    red_nl_o = ps_small.tile([1, NP], F32, tag="ps_sm")
    nc.tensor.matmul(red_ml_o[:], ones_bot[:], mlm[:], start=True, stop=True)
    nc.tensor.matmul(
        red_nl_o[:], ones_bot[:], masklT_pair[:], start=True, stop=True
    )
    nc.vector.reciprocal(rl[:], red_nl_o[:])
    nc.vector.tensor_tensor(t2v[:, :, 1], red_ml_o[:], rl[:], op=TT.mult)

    out_sb = smalls.tile([1, B_LOC], F32, tag="out_sb")
    nc.vector.tensor_tensor(out_sb[:], t1[:], t2[:], op=TT.add)
    nc.sync.dma_start(out=out[:], in_=out_sb[:])


_CACHED_NC = None


def _get_nc():
    global _CACHED_NC
    if _CACHED_NC is None:
        from contextlib import ExitStack

        nc = bacc.Bacc(
            "TRN2", target_bir_lowering=False, debug=False, num_devices=N_CORES
        )
        video = nc.dram_tensor(
            "video", [B_LOC, TV, D], F32, kind="ExternalInput"
        ).ap()
        lang = nc.dram_tensor("lang", [B_LOC, TL, D], F32, kind="ExternalInput").ap()
        mask_v = nc.dram_tensor(
            "mask_v", [B_LOC, TV], F32, kind="ExternalInput"
        ).ap()
        mask_l = nc.dram_tensor(
            "mask_l", [B_LOC, TL], F32, kind="ExternalInput"
        ).ap()
        out = nc.dram_tensor("out", [1, B_LOC], F32, kind="ExternalOutput").ap()
        with tile.TileContext(nc) as tc:
            with ExitStack() as ctx:
                _emit(nc, tc, ctx, video, lang, mask_v, mask_l, out)
        nc.compile()
        _CACHED_NC = nc
    return _CACHED_NC


def _run(video_feat, lang_feat, mask_v, mask_l, trace=False):
    nc = _get_nc()
    video_feat = np.ascontiguousarray(video_feat, dtype=np.float32)
    lang_feat = np.ascontiguousarray(lang_feat, dtype=np.float32)
    mask_v = np.ascontiguousarray(mask_v, dtype=np.float32)
    mask_l = np.ascontiguousarray(mask_l, dtype=np.float32)
    in_maps = []
    for c in range(N_CORES):
        sl = slice(c * B_LOC, (c + 1) * B_LOC)
        in_maps.append(
            {
                "video": video_feat[sl],
                "lang": lang_feat[sl],
                "mask_v": mask_v[sl],
                "mask_l": mask_l[sl],
            }
        )
    res = run_bass_kernel_spmd(nc, in_maps, list(range(N_CORES)), trace=trace)
    full = np.concatenate(
        [res.results[c]["out"].reshape(-1) for c in range(N_CORES)]
    ).astype(np.float32)
    return full, res


def kernel(video_feat, lang_feat, mask_v, mask_l):
    out, _ = _run(video_feat, lang_feat, mask_v, mask_l, trace=False)
    return out


# revision 52
# speedup vs baseline: 1.1087x; 1.1087x over previous
"""DoubleMaskedChamferDistance Trainium2 kernel.

Full inputs: video_feat [128,512,512] f32, lang_feat [128,64,512] f32,
mask_v [128,512] f32, mask_l [128,64] f32  ->  out [128] f32.

Sharding: data-parallel over batch B=128 across 8 cores (16 per core).

Math notes:
 - pd[v,l] = |v|^2 - 2 v.l + |l|^2 ; masked = pd + (1 - mask_v mask_l) * max(pd).
   Any constant M >= max(pd) shields identically (pd <= ~1400; M = 32768 = 2^15,
   exact in bf16/fp32). This removes the cross-batch/cross-core dependency.
 - Per batch pair, one PSUM accumulation in [(t,l), v] layout:
       psum[l,v] = -M*mask_l[l]*mask_v[v]  (one K=2 block-diag matmul per pair)
                 - 2*ab[l,v]               (4 bf16 matmuls per batch over d-chunks)
                 + a[v]                    (1 all-ones matmul per batch)
   and + (b[l] + M) is applied as the ACT bias at evacuation.
 - minsl = min over v: free-dim reduce of masked. minsv = min over l:
   PE-transpose masked to [v, (t,l)] strips, free-dim reduce.
 - Per-batch partition sums are deferred: minsv/minsl/mask columns are
   collected across the batch loop and reduced once at the end (ones-matmuls).
 - DMA head: video chunk DMAs are issued first (gpsimd SWDGE), lang is
   split per-pair and interleaved, masks ride the sync HWDGE queue, and
   identity generation is emitted between the first chunk issues.

Toolchain constraint honored throughout: every DMA instruction may carry at
most ONE semaphore wait, so DMAs only ever write fresh (never-recycled) tiles
and all data marshalling between tiles is done by compute engines.
"""

import numpy as np

import concourse.bass as bass
import concourse.mybir as mybir
import concourse.tile as tile
from concourse import bacc, masks
from concourse.bass_utils import run_bass_kernel_spmd

N_CORES = 8
B, TV, TL, D = 128, 512, 64, 512
B_LOC = B // N_CORES  # 16
M_CONST = 32768.0

F32 = mybir.dt.float32
BF16 = mybir.dt.bfloat16
AX = mybir.AxisListType


def _emit(nc, tc, ctx, video, lang, mask_v, mask_l, out):
    TT = mybir.AluOpType
    AF = mybir.ActivationFunctionType

    consts = ctx.enter_context(tc.tile_pool(name="consts", bufs=1))
    vnat = ctx.enter_context(tc.tile_pool(name="vnat", bufs=1))
    vT = ctx.enter_context(tc.tile_pool(name="vT", bufs=6))
    langfresh = ctx.enter_context(tc.tile_pool(name="langfresh", bufs=1))
    langp = ctx.enter_context(tc.tile_pool(name="langp", bufs=3))
    sqs = ctx.enter_context(tc.tile_pool(name="sqs", bufs=2))
    smalls = ctx.enter_context(tc.tile_pool(name="smalls", bufs=4))
    maskedp = ctx.enter_context(tc.tile_pool(name="maskedp", bufs=3))
    ps_vT = ctx.enter_context(tc.tile_pool(name="ps_vT", bufs=2, space="PSUM"))
    ps_main = ctx.enter_context(tc.tile_pool(name="ps_main", bufs=2, space="PSUM"))
    ps_small = ctx.enter_context(tc.tile_pool(name="ps_small", bufs=2, space="PSUM"))

    NP = B_LOC // 2  # batch pairs

    # ---- DMA issue order is the head-latency critical path: video first ----
    vchunks = []
    lang_pairs = [None] * NP

    def issue_vchunk(c):
        t = vnat.tile([128, 4, 512], BF16, tag=f"vch{c}")
        nc.gpsimd.dma_start(
            out=t[:], in_=video[c].rearrange("(s p) d -> p s d", p=128)
        )
        vchunks.append(t)

    def issue_lang(j0, nj):
        # one DMA covers pairs [j0, j0+nj): [128(two,l), nj, 512] bf16
        t = langfresh.tile([128, nj, 512], BF16, tag=f"lg{j0}")
        nc.gpsimd.dma_start(
            out=t[:],
            in_=lang[2 * j0 : 2 * (j0 + nj)].rearrange(
                "(j two) l d -> (two l) j d", two=2
            ),
        )
        for jj in range(nj):
            lang_pairs[j0 + jj] = t[:, jj]

    # SWDGE assigns issues to 8 rotating rings in order, and ring service
    # order is not FIFO across rings: anything issued late can see its
    # transfers deferred past the whole video stream. So: small early-needed
    # operands go first, then all 16 video chunks back-to-back. The negm
    # memset runs on gpsimd so its DMAs' wait resolves in-queue instead of
    # stalling descriptor generation on a cross-engine semaphore.
    negm = consts.tile([2, NP, 128], BF16)
    nc.gpsimd.memset(negm[:], 0.0)

    issue_lang(0, 4)
    issue_lang(4, 4)
    nc.gpsimd.dma_start(
        out=negm[0:1, :, 0:64],
        in_=mask_l.rearrange("(j two) l -> two j l", two=2)[0:1],
    )
    nc.gpsimd.dma_start(
        out=negm[1:2, :, 64:128],
        in_=mask_l.rearrange("(j two) l -> two j l", two=2)[1:2],
    )
    # mask_v in pair layout for the K=2 mask matmul: mv_pairs[t, j, v]
    # (bf16 cast -> gpsimd SWDGE)
    mv_pairs = consts.tile([2, NP, 512], BF16)
    nc.gpsimd.dma_start(
        out=mv_pairs[:], in_=mask_v.rearrange("(j two) v -> two j v", two=2)
    )
    mvn = consts.tile([2, NP, 512], BF16)
    nc.vector.tensor_scalar_mul(mvn[:], mv_pairs[:], -M_CONST)

    # identb on gpsimd (affine_select is gpsimd-only); identf derived from
    # it by a DVE cast copy so gpsimd only pays for one identity build
    identb = consts.tile([128, 128], BF16)
    masks.make_identity(nc, identb[:])
    identf = consts.tile([128, 128], F32)
    nc.vector.tensor_copy(identf[:], identb[:])

    for c in range(B_LOC):
        issue_vchunk(c)

    # masks ride the sync (HWDGE) queue in parallel with the video stream
    maskv_nat = consts.tile([B_LOC, 512], F32)
    nc.sync.dma_start(out=maskv_nat[:], in_=mask_v)
    maskl_pair_nat = consts.tile([NP, 128], F32)
    nc.sync.dma_start(
        out=maskl_pair_nat[:], in_=mask_l.rearrange("(j two) l -> j (two l)", two=2)
    )

    # ---- constants / mask prep (compute engines, under the DMA stream) ----
    ones128 = consts.tile([128, 1], F32)
    nc.vector.memset(ones128[:], 1.0)
    ones_mat = consts.tile([128, 64], BF16)
    nc.vector.memset(ones_mat[:], 1.0)
    ones_top = consts.tile([128, 1], F32)
    nc.vector.memset(ones_top[:], 0.0)
    nc.vector.memset(ones_top[0:64], 1.0)
    ones_bot = consts.tile([128, 1], F32)
    nc.vector.memset(ones_bot[:], 0.0)
    nc.vector.memset(ones_bot[64:128], 1.0)

    # mask_v columns for the final masked sums: [v%128, v//128, b]
    mvc_ps = ps_small.tile([128, 4, B_LOC], F32, tag="ps_sm")
    for s in range(4):
        nc.tensor.transpose(
            mvc_ps[:, s],
            maskv_nat[:, 128 * s : 128 * (s + 1)],
            identf[0:B_LOC, 0:B_LOC],
        )
    maskv_cols = consts.tile([128, 4, B_LOC], F32)
    nc.vector.tensor_copy(maskv_cols[:], mvc_ps[:])

    # masklT_pair[(two l), j] = mask_l[2j + two, l]
    mlc_ps = ps_small.tile([128, NP], F32, tag="ps_sm")
    nc.tensor.transpose(mlc_ps[:], maskl_pair_nat[:], identf[0:NP, 0:NP])
    masklT_pair = consts.tile([128, NP], F32)
    nc.vector.tensor_copy(masklT_pair[:], mlc_ps[:])

    # collectors (written per pair/batch, reduced once at the end).
    # minsv_all is s-major [v%128, v//128, b] so the per-pair reduce's
    # output slice is packed (keeps the DVE 2x perf mode).
    minsv_all = consts.tile([128, 4, B_LOC], BF16)
    minsl_pairs = consts.tile([128, NP], F32)
    b_pairs = consts.tile([128, NP], F32)
    bias_pairs = consts.tile([128, NP], F32)

    # ---- all lang-side prep happens up front, in the window where the PE
    # is otherwise idle waiting for the first video chunks ----
    langTs = []
    for j in range(NP):
        lg = lang_pairs[j]
        sq_l = sqs.tile([128, 512], BF16, tag="sq_l")
        nc.scalar.activation(
            sq_l[:], lg[:], AF.Square, accum_out=b_pairs[:, j : j + 1]
        )
        lg_ps = ps_small.tile([128, 4, 128], BF16, tag="ps_sm")
        for k in range(4):
            nc.tensor.transpose(
                lg_ps[:, k], lg[:, 128 * k : 128 * (k + 1)], identb[:]
            )
        langT = langp.tile([128, 4, 128], BF16, tag=f"langT{j}")
        nc.vector.tensor_scalar_mul(langT[:], lg_ps[:], -2.0)
        langTs.append(langT)
    nc.vector.tensor_scalar_add(bias_pairs[:], b_pairs[:], M_CONST)

    # Software pipeline, one pair deep: prep(j) = PE transposes + DVE/ACT
    # evacuations + squares; compute(j) = matmuls + masked evac + mins.
    # Emitting compute(j-1) AFTER prep(j) keeps the in-order PE queue from
    # head-blocking on pair j-1's DVE/ACT chain: the PE runs pair j's
    # transposes while pair j-1's sq1/langT dependencies settle.
    prepped = {}

    def prep(j):
        vt_sbs, sq1s = [], []
        for t in range(2):
            vstrip = vchunks[2 * j + t]  # [128, 4, 512] bf16 (p, s, d)
            # videoT transposes; evacuations split DVE/ACT
            vt_sb = vT.tile([128, 4, 512], BF16, tag="vt_sb")
            vt_ps = ps_vT.tile([128, 4, 512], BF16, tag="vt_ps")
            for k in range(4):
                for s in range(4):
                    nc.tensor.transpose(
                        vt_ps[:, k, 128 * s : 128 * (s + 1)],
                        vstrip[:, s, 128 * k : 128 * (k + 1)],
                        identb[:],
                    )
            nc.scalar.copy(vt_sb[:], vt_ps[:])

            # a-chunks: vt^2 pre-added to one [128,512] slab (DVE).
            # (tensor_tensor may read at most ONE operand from PSUM, so the
            # squares read the SBUF copy.)
            sq4 = sqs.tile([128, 4, 512], BF16, tag="sq4")
            nc.vector.tensor_tensor(sq4[:], vt_sb[:], vt_sb[:], op=TT.mult)
            sq2 = sqs.tile([128, 2, 512], BF16, tag="sq2")
            nc.vector.tensor_tensor(sq2[:], sq4[:, 0:2], sq4[:, 2:4], op=TT.add)
            sq1 = sqs.tile([128, 512], BF16, tag=f"sq1_{t}")
            nc.vector.tensor_tensor(sq1[:], sq2[:, 0], sq2[:, 1], op=TT.add)
            vt_sbs.append(vt_sb)
            sq1s.append(sq1)
        prepped[j] = (vt_sbs, sq1s)

    def compute(j):
        vt_sbs, sq1s = prepped.pop(j)
        langT = langTs[j]
        # PSUM: per-half ab (k=0 opens) + a, then one K=2 mask matmul
        # (full pair width) closes the accumulation
        psum_pair = ps_main.tile([128, 512], F32, tag="psum_T")
        for t in range(2):
            half = psum_pair[64 * t : 64 * (t + 1), :]
            vt_sb = vt_sbs[t]
            for k in range(4):
                nc.tensor.matmul(
                    half,
                    langT[:, k, 64 * t : 64 * (t + 1)],
                    vt_sb[:, k],
                    start=(k == 0),
                    stop=False,
                    skip_group_check=True,
                )
            nc.tensor.matmul(
                half,
                ones_mat[:],
                sq1s[t][:],
                start=False,
                stop=False,
                skip_group_check=True,
            )
        nc.tensor.matmul(
            psum_pair[:],
            negm[:, j],
            mvn[:, j],
            start=False,
            stop=True,
            skip_group_check=True,
        )

        # masked evacuation with +(b + M) bias (bf16), both batches
        masked_pr = maskedp.tile([128, 512], BF16, tag="masked_pr")
        nc.scalar.activation(
            masked_pr[:],
            psum_pair[:],
            AF.Identity,
            bias=bias_pairs[:, j : j + 1],
            scale=1.0,
        )

        # minsl: min over v (free dim), both batches at once
        nc.vector.tensor_reduce(
            minsl_pairs[:, j : j + 1], masked_pr[:], axis=AX.X, op=TT.min
        )

        # minsv: transpose [128,128] pair-blocks, min over l
        o2 = ps_small.tile([128, 4, 2, 64], BF16, tag="ps_sm")
        for s in range(4):
            nc.tensor.transpose(
                o2[:, s], masked_pr[:, 128 * s : 128 * (s + 1)], identb[:]
            )
        nc.vector.tensor_reduce(
            minsv_all[:, :, 2 * j : 2 * j + 2], o2[:], axis=AX.X, op=TT.min
        )

    for j in range(NP):
        prep(j)
        if j >= 1:
            compute(j - 1)
    compute(NP - 1)

    # ---- final: masked sums via ones-matmuls over collected columns ----
    mv_mask = consts.tile([128, 4, B_LOC], F32)
    nc.vector.tensor_tensor(mv_mask[:], minsv_all[:], maskv_cols[:], op=TT.mult)
    mv_sums = consts.tile([128, B_LOC], F32)
    nc.vector.tensor_reduce(
        mv_sums[:], mv_mask[:].rearrange("p s b -> p b s"), axis=AX.X, op=TT.add
    )
    nv_sums = consts.tile([128, B_LOC], F32)
    nc.vector.tensor_reduce(
        nv_sums[:],
        maskv_cols[:].rearrange("p s b -> p b s"),
        axis=AX.X,
        op=TT.add,
    )
    mlm = consts.tile([128, NP], F32)
    nc.vector.tensor_tensor(mlm[:], minsl_pairs[:], masklT_pair[:], op=TT.mult)

    red_mv = ps_main.tile([1, B_LOC], F32, tag="psum_T")
    red_nv = ps_small.tile([1, B_LOC], F32, tag="ps_sm")
    nc.tensor.matmul(red_mv[:], ones128[:], mv_sums[:], start=True, stop=True)
    nc.tensor.matmul(red_nv[:], ones128[:], nv_sums[:], start=True, stop=True)
    rv = smalls.tile([1, B_LOC], F32, tag="rv")
    t1 = smalls.tile([1, B_LOC], F32, tag="t1")
    nc.vector.reciprocal(rv[:], red_nv[:])
    nc.vector.tensor_tensor(t1[:], red_mv[:], rv[:], op=TT.mult)

    # even/odd batch reductions as separate partition-0 matmuls, written
    # into the interleaved positions of t2 via strided views
    t2 = smalls.tile([1, B_LOC], F32, tag="t2")
    t2v = t2[:].rearrange("a (jj two) -> a jj two", two=2)
    rl = smalls.tile([1, NP], F32, tag="rl")

    red_ml_e = ps_main.tile([1, NP], F32, tag="psum_T")
    red_nl_e = ps_small.tile([1, NP], F32, tag="ps_sm")
    nc.tensor.matmul(red_ml_e[:], ones_top[:], mlm[:], start=True, stop=True)
    nc.tensor.matmul(
        red_nl_e[:], ones_top[:], masklT_pair[:], start=True, stop=True
    )
    nc.vector.reciprocal(rl[:], red_nl_e[:])
    nc.vector.tensor_tensor(t2v[:, :, 0], red_ml_e[:], rl[:], op=TT.mult)

    red_ml_o = ps_main.tile([1, NP], F32, tag="psum_T")
    red_nl_o = ps_small.tile([1, NP], F32, tag="ps_sm")
    nc.tensor.matmul(red_ml_o[:], ones_bot[:], mlm[:], start=True, stop=True)
    nc.tensor.matmul(
        red_nl_o[:], ones_bot[:], masklT_pair[:], start=True, stop=True
    )
    nc.vector.reciprocal(rl[:], red_nl_o[:])
    nc.vector.tensor_tensor(t2v[:, :, 1], red_ml_o[:], rl[:], op=TT.mult)

    out_sb = smalls.tile([1, B_LOC], F32, tag="out_sb")
    nc.vector.tensor_tensor(out_sb[:], t1[:], t2[:], op=TT.add)
    nc.sync.dma_start(out=out[:], in_=out_sb[:])


_CACHED_NC = None


def _get_nc():
    global _CACHED_NC
    if _CACHED_NC is None:
        from contextlib import ExitStack

        nc = bacc.Bacc(
            "TRN2", target_bir_lowering=False, debug=False, num_devices=N_CORES
        )
        video = nc.dram_tensor(
            "video", [B_LOC, TV, D], F32, kind="ExternalInput"
        ).ap()
        lang = nc.dram_tensor("lang", [B_LOC, TL, D], F32, kind="ExternalInput").ap()
        mask_v = nc.dram_tensor(
            "mask_v", [B_LOC, TV], F32, kind="ExternalInput"
        ).ap()
        mask_l = nc.dram_tensor(
            "mask_l", [B_LOC, TL], F32, kind="ExternalInput"
        ).ap()
        out = nc.dram_tensor("out", [1, B_LOC], F32, kind="ExternalOutput").ap()
        with tile.TileContext(nc) as tc:
            with ExitStack() as ctx:
                _emit(nc, tc, ctx, video, lang, mask_v, mask_l, out)
        nc.compile()
        _CACHED_NC = nc
    return _CACHED_NC


def _run(video_feat, lang_feat, mask_v, mask_l, trace=False):
    nc = _get_nc()
    video_feat = np.ascontiguousarray(video_feat, dtype=np.float32)
    lang_feat = np.ascontiguousarray(lang_feat, dtype=np.float32)
    mask_v = np.ascontiguousarray(mask_v, dtype=np.float32)
    mask_l = np.ascontiguousarray(mask_l, dtype=np.float32)
    in_maps = []
    for c in range(N_CORES):
        sl = slice(c * B_LOC, (c + 1) * B_LOC)
        in_maps.append(
            {
                "video": video_feat[sl],
                "lang": lang_feat[sl],
                "mask_v": mask_v[sl],
                "mask_l": mask_l[sl],
            }
        )
    res = run_bass_kernel_spmd(nc, in_maps, list(range(N_CORES)), trace=trace)
    full = np.concatenate(
        [res.results[c]["out"].reshape(-1) for c in range(N_CORES)]
    ).astype(np.float32)
    return full, res


def kernel(video_feat, lang_feat, mask_v, mask_l):
    out, _ = _run(video_feat, lang_feat, mask_v, mask_l, trace=False)
    return out
